# revision 51
# baseline (speedup 1.0000x reference)
"""Trainium2 Bass kernel for nn_KGICLPromptEnhancer (optimized).

Reference computation (B=256, R=2048, H=64, E=20):
  rel_emb[b,r] = (r==query[b]) ? ones : 0.1*init_noise[b,r]
  h = rel_emb[b, edge_type[b,e]]                        (gather)
  msg = relu([h,h] @ msg_W + msg_b)                     = relu(h @ (msg_W[:H]+msg_W[H:]) + msg_b)
  agg = segment_sum(msg, edge_type, R)                  (scatter-add, <=20 touched rows)
  prompt = LN(agg @ upd_W + upd_b) * ln_g + ln_b
  combined = [base, prompt]
  fused = relu(combined @ fus_W1 + fus_b1) @ fus_W2 + fus_b2
  gate = sigmoid(combined @ gate_W + gate_b)
  out = gate * fused + (1-gate) * base

Structure:
  * agg==0 for untouched relations -> prompt == pz (a host constant) except
    on the <=20 touched rows per sample.  Never streams init_noise: gathers
    only the needed rows (indirect DMA).
  * Duplicate edges (same sample+relation) carry IDENTICAL messages, so
    segment_sum == count * msg: no duplicate-resolution matmul needed.
    Host precomputes counts, masks, gather indices, and per-pair one-hot
    scatter matrices from the integer inputs.
  * Phase A (per-edge prompt deltas) is batched across pairs in 6 groups
    of <=120 edges; one shared Sqrt for all groups keeps the ACT engine on
    a single table set (sqrt loads once, sigmoid set loads once).
  * Phase B streams base feature-major (2 samples x 64 feats = 128
    partitions, R columns) in bf16, applies the block-diagonal fused MLP +
    gate with deltas folded in via one-hot matmuls, and writes bf16 out.
  * Elementwise combine is split DVE/GpSimd to balance engines.

Memory floor per core (bf16): read base 8.4MB + write out 8.4MB.
"""

import numpy as np

import concourse.bass as bass
import concourse.tile as tile
from concourse import mybir
from concourse.bass_utils import run_bass_kernel_spmd

B, R, H, E = 256, 2048, 64, 20
LN_EPS = 1e-5
N_CORES = 8
SPC = B // N_CORES          # samples per core = 32
PAIRS = SPC // 2            # sample pairs per core = 16
EP = 2 * E                  # edges per pair = 40
NEDGE = SPC * E             # edges per core = 640
GSIZE = 3 * EP              # edges per phase-A group = 120
NG = (NEDGE + GSIZE - 1) // GSIZE   # 6 groups (5 full + 1 of 40)
CHUNK = 512
NCHUNK = R // CHUNK

F32 = mybir.dt.float32
BF16 = mybir.dt.bfloat16
I32 = mybir.dt.int32

# Set by test.py to capture an NTFF profile (prints HW exec time).
PROFILE = False
LAST_EXEC_NS = None


def _split_multi_waits(nc, max_waits=1):
    """This walrus build rejects instructions carrying more than one sync
    wait. Hoist extra waits onto no-op instructions on the same engine
    immediately before the over-subscribed instruction."""
    k = 0
    for f in nc.m.functions:
        for bb in f.blocks:
            out = []
            for inst in bb.instructions:
                si = inst.sync_info
                if si is not None and len(si.on_wait) > max_waits:
                    keep = list(si.on_wait[-max_waits:])
                    for w in si.on_wait[:-max_waits]:
                        k += 1
                        out.append(mybir.InstNoOp(
                            name=f"I-wsplit-{k}",
                            engine=inst.engine,
                            sync_info=mybir.SyncInfo(on_wait=[w], on_update=[]),
                        ))
                    del si.on_wait[:]
                    si.on_wait.extend(keep)
                out.append(inst)
            bb.instructions[:] = out


def _bf(x):
    import ml_dtypes
    return np.ascontiguousarray(np.asarray(x, dtype=np.float32)).astype(ml_dtypes.bfloat16)


def _f8(x):
    import ml_dtypes
    return np.ascontiguousarray(np.asarray(x, dtype=np.float32)).astype(
        ml_dtypes.float8_e4m3fn)


# fp8 weights are stored x8 to stay in e4m3's normal range; the inverse
# 1/8 rides the downstream activation's `scale` input for free.
F8S = 8.0

# Packed-constant layout: name -> (blob, col offset, rows, cols).  All
# constants ride in four dtype-segregated [128, N] blobs (one DMA each).
_CONST_SPEC = [
    ("e_idx", "cI32", GSIZE, NG),
    # cF32: phase-A critical path constants (small, lands ~3us)
    ("ident", "cF32", 128, 128),
    ("c1_blk", "cF32", 128, 1),
    ("b2_blk", "cF32", 128, 1),
    ("cg_col", "cF32", 128, 1),
    ("eps_col", "cF32", GSIZE, 1),
    ("e_a", "cF32", GSIZE, NG),
    ("e_m", "cF32", GSIZE, NG),
    ("e_cnt", "cF32", GSIZE, NG),
    ("e_rinv", "cF32", GSIZE, NG),
    ("e_pzr", "cF32", GSIZE, NG * H),
    # cF32L: bulky / tail-only constants (separate queue)
    ("g_bc", "cF32L", GSIZE, H),
    ("e_lbr", "cF32L", GSIZE, NG * H),
    ("p_maskA", "cF32L", EP, PAIRS),
    ("p_maskB", "cF32L", EP, PAIRS),
    ("p_mAB", "cF32L", EP, PAIRS * 128),
    ("negI", "cBF16", 128, 128),
    ("Weff_aug", "cBF16", H + 1, H),
    ("updW_aug", "cBF16", H + 1, H),
    ("W2_blk", "cBF16", 128, 128),
    ("W1bG", "cBF16", H, H + 1),
    ("W1a_f8", "cF8", 128, 128),
    ("Ga_f8", "cF8", 128, 128),
]
_BLOB_COLS = {}
_CONST_OFF = {}
for _n, _b, _r, _c in _CONST_SPEC:
    _CONST_OFF[_n] = (_b, _BLOB_COLS.get(_b, 0), _r, _c)
    _BLOB_COLS[_b] = _BLOB_COLS.get(_b, 0) + _c


def _pack_consts(cdict):
    import ml_dtypes
    npdt = {"cI32": np.int32, "cF32": np.float32, "cF32L": np.float32,
            "cBF16": ml_dtypes.bfloat16, "cF8": ml_dtypes.float8_e4m3fn}
    out = {b: np.zeros((128, n), npdt[b]) for b, n in _BLOB_COLS.items()}
    for name, (b, off, r, c) in _CONST_OFF.items():
        arr = np.asarray(cdict[name])
        assert arr.shape == (r, c), (name, arr.shape, (r, c))
        out[b][0:r, off:off + c] = arr
    return out


def _consts(w):
    """Weight-derived constants (weights replicated across cores)."""
    msg_W, msg_b = w["msg_W"], w["msg_b"]
    upd_W, upd_b = w["upd_W"], w["upd_b"]
    ln_g, ln_b = w["ln_g"], w["ln_b"]
    fus_W1, fus_b1 = w["fus_W1"], w["fus_b1"]
    fus_W2, fus_b2 = w["fus_W2"], w["fus_b2"]
    gate_W, gate_b = w["gate_W"], w["gate_b"]

    W_eff = msg_W[:H] + msg_W[H:]                                   # [64,64]
    Weff_aug = np.concatenate([W_eff, msg_b[None, :]], 0)           # [65,64]
    updW_aug = np.concatenate([upd_W, upd_b[None, :]], 0)           # [65,64]

    # prompt for untouched rows: LN(upd_b)*g + b
    u = upd_b.astype(np.float64)
    mu, var = u.mean(), u.var()
    pz = ((u - mu) / np.sqrt(var + LN_EPS) * ln_g + ln_b).astype(np.float32)  # [64]

    c1 = pz @ fus_W1[H:] + fus_b1                                   # [64]
    cg = float(pz @ gate_W[H:, 0] + gate_b[0])

    W1a_blk = np.zeros((128, 128), np.float32)
    W1a_blk[:64, :64] = fus_W1[:H]
    W1a_blk[64:, 64:] = fus_W1[:H]
    W2_blk = np.zeros((128, 128), np.float32)
    W2_blk[:64, :64] = fus_W2
    W2_blk[64:, 64:] = fus_W2
    Ga_rep = np.zeros((128, 128), np.float32)
    Ga_rep[:64, :64] = np.tile(gate_W[:H, 0][:, None], (1, 64))
    Ga_rep[64:, 64:] = np.tile(gate_W[:H, 0][:, None], (1, 64))
    W1bG = np.concatenate([fus_W1[H:], gate_W[H:]], 1)              # [64,65]

    c = {
        "ident": np.eye(128, dtype=np.float32),
        "negI": _bf(-np.eye(128, dtype=np.float32)),
        "Weff_aug": _bf(Weff_aug),
        "updW_aug": _bf(updW_aug),
        "W1a_f8": _f8(W1a_blk * F8S),
        "W2_blk": _bf(W2_blk),
        "Ga_f8": _f8(Ga_rep * F8S),
        "W1bG": _bf(W1bG),
        "c1_blk": np.tile(c1.astype(np.float32), 2)[:, None],       # [128,1]
        "b2_blk": np.tile(fus_b2.astype(np.float32), 2)[:, None],   # [128,1]
        "cg_col": np.full((128, 1), cg, np.float32),
        "eps_col": np.full((GSIZE, 1), LN_EPS, np.float32),
        "g_bc": np.tile(ln_g.astype(np.float32), (GSIZE, 1)),       # [120,64]
    }
    meta = {
        "pz": pz,
        "ln_b": ln_b.astype(np.float32),
        "has_b2": bool(np.any(fus_b2)),
        "has_g": bool(np.any(ln_g != 1.0)),
        "has_lnb": bool(np.any(ln_b)),
    }
    return c, meta


def _edge_consts(qr, et, meta):
    """Per-edge constants derived from the integer inputs (per core).

    qr: [SPC] int32, et: [SPC, E] int32.  Edge order: flat (sample, e).
    """
    pz, ln_b = meta["pz"], meta["ln_b"]
    s_of_e = np.repeat(np.arange(SPC), E)                 # [640]
    etf = et.reshape(NEDGE)                               # [640]
    idx = (s_of_e * R + etf).astype(np.int32)             # noise row gather
    is_q = (etf == qr[s_of_e]).astype(np.float32)         # query-relation mask
    a = 0.1 * (1.0 - is_q)                                # h = a*noise + m
    # duplicate count of (sample, relation) among the sample's edges
    cnt = np.zeros(NEDGE, np.float32)
    for s in range(SPC):
        vals, inv, c = np.unique(et[s], return_inverse=True, return_counts=True)
        cnt[s * E:(s + 1) * E] = c[inv]
    rinv = 1.0 / cnt
    # block placement: sample parity within its pair
    parity = (s_of_e % 2).astype(np.float32)              # 0 = A, 1 = B
    maskA = 1.0 - parity
    maskB = parity

    pad = NG * GSIZE - NEDGE                              # pad to 720

    def padv(x):
        return np.concatenate([x, np.zeros((pad,) + x.shape[1:], x.dtype)])

    def cols(x):                                          # [720] -> [120, NG]
        return np.ascontiguousarray(padv(x).reshape(NG, GSIZE).T)

    ec = {
        "e_idx": cols(idx).astype(np.int32),              # [120, NG] i32
        "e_a": cols(a), "e_m": cols(is_q),
        "e_cnt": cols(cnt), "e_rinv": cols(rinv),
        # per-pair layouts (base partition 0): [EP, PAIRS]; x8 fp8 scale
        "p_maskA": np.ascontiguousarray(F8S * maskA.reshape(PAIRS, EP).T),
        "p_maskB": np.ascontiguousarray(F8S * maskB.reshape(PAIRS, EP).T),
        # pz*rinv rows, per group: [120, NG*64]
        "e_pzr": np.ascontiguousarray(
            padv(rinv[:, None] * pz[None, :]).reshape(NG, GSIZE, H)
            .transpose(1, 0, 2).reshape(GSIZE, NG * H)),
        # ln_b*rinv rows (only used if has_lnb)
        "e_lbr": np.ascontiguousarray(
            padv(rinv[:, None] * ln_b[None, :]).reshape(NG, GSIZE, H)
            .transpose(1, 0, 2).reshape(GSIZE, NG * H)),
        # maskAB for gate-delta replication, per pair: [EP, PAIRS*128]; x8
        "p_mAB": np.ascontiguousarray(
            F8S * np.concatenate([np.tile(maskA[:, None], (1, 64)),
                                  np.tile(maskB[:, None], (1, 64))], 1)
            .reshape(PAIRS, EP, 128).transpose(1, 0, 2).reshape(EP, PAIRS * 128)),
    }
    # one-hot scatter matrices, per pair, padded to 128 k-rows (DoubleRow
    # k-subtile 1): [PAIRS, 128, R] f32
    oh = np.zeros((PAIRS, 128, R), np.float32)
    oh[np.repeat(np.arange(PAIRS), EP),
       np.tile(np.arange(EP), PAIRS),
       etf.reshape(PAIRS, EP).reshape(-1)] = 1.0
    ec["ohpad"] = oh
    return ec


def _gsz(g):
    return min(GSIZE, NEDGE - g * GSIZE)


def _build_program(meta, split_waits=True):
    """Trace the SPMD Bass program (identical for all cores)."""
    nc = bass.Bass()

    F8 = mybir.dt.float8e4
    baseT = nc.dram_tensor("baseT", [PAIRS, 128, R], BF16, kind="ExternalInput")
    rhsID = nc.dram_tensor("rhsI", [PAIRS, 128, 2, R], F8, kind="ExternalInput")
    noise = nc.dram_tensor("noise", [SPC * R, H], F32, kind="ExternalInput")
    outT = nc.dram_tensor("outT", [PAIRS, 128, R], BF16, kind="ExternalOutput")

    # Constants arrive packed per dtype (one DMA each instead of ~23 small
    # serialized DMAs at startup); ct[k] views slice the packed tiles.
    blobs = {"cI32": I32, "cF32": F32, "cF32L": F32, "cBF16": BF16, "cF8": F8}
    bdram = {b: nc.dram_tensor(b, [128, _BLOB_COLS[b]], dt, kind="ExternalInput")
             for b, dt in blobs.items()}

    AF = mybir.ActivationFunctionType
    OP = mybir.AluOpType

    with tile.TileContext(nc) as tc:
        with (
            tc.tile_pool(name="consts", bufs=1) as cp,
            tc.tile_pool(name="pa_sb", bufs=3) as pa,
            tc.tile_pool(name="pa_keep", bufs=1) as pk,
            tc.tile_pool(name="pa_ps", bufs=2, space="PSUM") as pap,
            tc.tile_pool(name="oh_sb", bufs=4) as poh,
            tc.tile_pool(name="pb_in", bufs=3) as pbi,
            tc.tile_pool(name="pb_sb", bufs=2) as pb,
            tc.tile_pool(name="pb_ch", bufs=3) as pc2,
            tc.tile_pool(name="ps_z1", bufs=2, space="PSUM") as pz1,
            tc.tile_pool(name="ps_f", bufs=2, space="PSUM") as pf,
            tc.tile_pool(name="ps_g", bufs=2, space="PSUM") as pg,
        ):
            # ---- constants: 5 packed DMAs spread over 3 issue queues so
            # the phase-A-critical blobs land within ~3.5us ----
            dma_eng = {"cI32": nc.sync, "cF32": nc.sync, "cF8": nc.sync,
                       "cBF16": nc.gpsimd, "cF32L": nc.scalar}
            ctile = {}
            for b in ("cI32", "cF32", "cF8", "cBF16", "cF32L"):
                t = cp.tile([128, _BLOB_COLS[b]], blobs[b], name=b)
                dma_eng[b].dma_start(t[:], bdram[b][:, :])
                ctile[b] = t
            ct = {}
            for k, (b, off, r, c) in _CONST_OFF.items():
                ct[k] = ctile[b][0:r, off:off + c]

            # LN-variance tiles: column g = group g's stats
            var_all = cp.tile([GSIZE, NG], F32, name="var_all")
            std_all = cp.tile([GSIZE, NG], F32, name="std_all")
            rstd_all = cp.tile([GSIZE, NG], F32, name="rstd_all")

            # ================= phase A: per-edge prompt deltas =========
            xc_g, payload_g, dgrep_g = [], [], []
            for g in range(NG):
                n = _gsz(g)
                # gather the needed noise rows
                hraw = pa.tile([GSIZE, H], F32, tag="hraw")
                nc.gpsimd.indirect_dma_start(
                    out=hraw[0:n, :], out_offset=None, in_=noise[:, :],
                    in_offset=bass.IndirectOffsetOnAxis(
                        ap=ct["e_idx"][0:n, g:g + 1], axis=0))
                # h = a*noise + m  (query rows -> 1, else 0.1*noise)
                h = pa.tile([GSIZE, H + 1], F32, tag="h")
                nc.vector.tensor_scalar(h[0:n, 0:H], hraw[0:n, :],
                                        ct["e_a"][0:n, g:g + 1],
                                        ct["e_m"][0:n, g:g + 1],
                                        op0=OP.mult, op1=OP.add)
                nc.vector.memset(h[0:n, H:H + 1], 1.0)
                # msg = relu(h @ Weff + msg_b); agg = cnt * msg
                hT_ps = pap.tile([H + 1, GSIZE], F32, tag="aps", name=f"hT{g}")
                nc.tensor.transpose(hT_ps[:, 0:n], h[0:n, :], ct["ident"][0:n, 0:n])
                hT = pa.tile([H + 1, GSIZE], BF16, tag="hT")
                nc.vector.tensor_copy(hT[:, 0:n], hT_ps[:, 0:n])
                msg_ps = pap.tile([GSIZE, H], F32, tag="aps", name=f"msg{g}")
                nc.tensor.matmul(msg_ps[0:n, :], lhsT=hT[:, 0:n], rhs=ct["Weff_aug"][:])
                agg = pa.tile([GSIZE, H + 1], F32, tag="agg")
                nc.vector.tensor_scalar(agg[0:n, 0:H], msg_ps[0:n, :],
                                        0.0, ct["e_cnt"][0:n, g:g + 1],
                                        op0=OP.max, op1=OP.mult)
                nc.vector.memset(agg[0:n, H:H + 1], 1.0)
                # upd = agg @ updW + upd_b
                aggT_ps = pap.tile([H + 1, GSIZE], F32, tag="aps", name=f"aT{g}")
                nc.tensor.transpose(aggT_ps[:, 0:n], agg[0:n, :], ct["ident"][0:n, 0:n])
                aggT = pa.tile([H + 1, GSIZE], BF16, tag="aggT")
                nc.vector.tensor_copy(aggT[:, 0:n], aggT_ps[:, 0:n])
                upd_ps = pap.tile([GSIZE, H], F32, tag="aps", name=f"upd{g}")
                nc.tensor.matmul(upd_ps[0:n, :], lhsT=aggT[:, 0:n], rhs=ct["updW_aug"][:])
                # LN stats; xc kept until shared sqrt
                mu = pa.tile([GSIZE, 1], F32, tag="mu")
                nc.vector.reduce_sum(mu[0:n, :], upd_ps[0:n, :], axis=mybir.AxisListType.X)
                negmu = pa.tile([GSIZE, 1], F32, tag="negmu")
                nc.vector.tensor_scalar_mul(negmu[0:n, :], mu[0:n, :], -1.0 / H)
                xc = pk.tile([GSIZE, H], F32, tag=f"xc{g}")
                nc.vector.tensor_scalar_add(xc[0:n, :], upd_ps[0:n, :], negmu[0:n, :])
                sq = pa.tile([GSIZE, H], F32, tag="sq")
                nc.scalar.activation(sq[0:n, :], xc[0:n, :], AF.Square,
                                     accum_out=var_all[0:n, g:g + 1])
                # per-group sqrt: ACT's in-order queue runs all phase-A
                # table ops before phase B's first relu, so the sqrt ->
                # sigmoid table set switches twice total.
                nc.scalar.activation(std_all[0:n, g:g + 1], var_all[0:n, g:g + 1],
                                     AF.Sqrt, bias=ct["eps_col"][0:n, :],
                                     scale=1.0 / H)
                nc.vector.reciprocal(rstd_all[0:n, g:g + 1], std_all[0:n, g:g + 1])
                xc_g.append(xc)

            # Tails emitted after all fronts: keeps every phase-A Sqrt ahead
            # of every phase-B Relu/Sigmoid in the ACT queue (2 table loads
            # total).  Tails contain no ACT ops, so each group's tail still
            # runs as soon as its own front finishes.
            for g in range(NG):
                n = _gsz(g)
                xc = xc_g[g]
                # dls = (prompt - pz) * rinv = xc*(rstd*rinv)*ln_g - (pz-ln_b)*rinv
                rr = pa.tile([GSIZE, 1], F32, tag="rr")
                nc.vector.tensor_tensor(rr[0:n, :], rstd_all[0:n, g:g + 1],
                                        ct["e_rinv"][0:n, g:g + 1], op=OP.mult)
                dls = pa.tile([GSIZE, H], F32, tag="dls")
                if meta["has_g"]:
                    pn = pa.tile([GSIZE, H], F32, tag="pn")
                    nc.vector.tensor_scalar_mul(pn[0:n, :], xc[0:n, :], rr[0:n, :])
                    nc.vector.tensor_tensor(pn[0:n, :], pn[0:n, :], ct["g_bc"][0:n, :],
                                            op=OP.mult)
                    nc.vector.tensor_tensor(
                        dls[0:n, :], pn[0:n, :],
                        ct["e_pzr"][0:n, g * H:(g + 1) * H], op=OP.subtract)
                else:
                    nc.vector.scalar_tensor_tensor(
                        dls[0:n, :], xc[0:n, :], rr[0:n, :],
                        ct["e_pzr"][0:n, g * H:(g + 1) * H],
                        op0=OP.mult, op1=OP.subtract)
                if meta["has_lnb"]:
                    nc.vector.tensor_tensor(
                        dls[0:n, :], dls[0:n, :],
                        ct["e_lbr"][0:n, g * H:(g + 1) * H], op=OP.add)
                # payload = [dls @ W1b | dls @ Gb], block-placed per sample.
                # Per-pair matmuls: the pair offset rides the lhsT FREE dim
                # (partition base of matmul operands must be 0/32/64).
                dT_ps = pap.tile([H, GSIZE], F32, tag="aps", name=f"dT{g}")
                nc.tensor.transpose(dT_ps[:, 0:n], dls[0:n, :], ct["ident"][0:n, 0:n])
                dT = pa.tile([H, GSIZE], BF16, tag="dT")
                nc.vector.tensor_copy(dT[:, 0:n], dT_ps[:, 0:n])
                for j in range(n // EP):
                    i = g * 3 + j                        # global pair index
                    p0 = j * EP
                    pW_ps = pap.tile([EP, H + 1], F32, tag="aps", name=f"pW{i}")
                    nc.tensor.matmul(pW_ps[:], lhsT=dT[:, p0:p0 + EP],
                                     rhs=ct["W1bG"][:])
                    # DoubleRow lhsT tiles [128, 2, 128] fp8:
                    #   k-subtile 0 = bulk weights (x8), 1 = edge payload (x8)
                    z1w = pk.tile([128, 2, 128], F8, tag=f"zw{i}")
                    nc.vector.memset(z1w[:, 1, :], 0.0)
                    nc.vector.tensor_copy(z1w[:, 0, :], ct["W1a_f8"][:])
                    nc.vector.tensor_scalar_mul(z1w[0:EP, 1, 0:H], pW_ps[:, 0:H],
                                                ct["p_maskA"][:, i:i + 1])
                    nc.vector.tensor_scalar_mul(z1w[0:EP, 1, H:2 * H], pW_ps[:, 0:H],
                                                ct["p_maskB"][:, i:i + 1])
                    gw = pk.tile([128, 2, 128], F8, tag=f"gw{i}")
                    nc.vector.memset(gw[:, 1, :], 0.0)
                    nc.vector.tensor_copy(gw[:, 0, :], ct["Ga_f8"][:])
                    nc.vector.tensor_scalar_mul(
                        gw[0:EP, 1, :], ct["p_mAB"][:, i * 128:(i + 1) * 128],
                        pW_ps[:, H:H + 1])
                    payload_g.append(z1w)
                    dgrep_g.append(gw)

            # ================= phase B: bulk fused MLP + gate ==========
            for i in range(PAIRS):
                pl = payload_g[i]
                dg = dgrep_g[i]

                base_h = pbi.tile([128, R], BF16, tag="base_h")
                nc.sync.dma_start(base_h[:], baseT[i, :, :])
                rhsI = poh.tile([128, 2, R], mybir.dt.float8e4, tag="rhsI")
                nc.sync.dma_start(rhsI[:], rhsID[i, :, :, :])
                out_t = pb.tile([128, R], BF16, tag="out_t")

                # DoubleRow fp8 folds bulk (k-subtile 0) + edge deltas
                # (k-subtile 1) into a single matmul per chunk; f - base
                # accumulates on PE via -I.
                for ch in range(NCHUNK):
                    sl = slice(ch * CHUNK, (ch + 1) * CHUNK)
                    zt = pz1.tile([128, CHUNK], F32, tag="z1")
                    nc.tensor.matmul(zt[:], lhsT=pl[:, 0:2, :], rhs=rhsI[:, 0:2, sl],
                                     perf_mode=mybir.MatmulPerfMode.DoubleRow)
                    rz = pc2.tile([128, CHUNK], BF16, tag="rz")
                    nc.scalar.activation(rz[:], zt[:], AF.Relu, bias=ct["c1_blk"][:],
                                         scale=1.0 / F8S)
                    gps = pg.tile([128, CHUNK], F32, tag="gps")
                    nc.tensor.matmul(gps[:], lhsT=dg[:, 0:2, :], rhs=rhsI[:, 0:2, sl],
                                     perf_mode=mybir.MatmulPerfMode.DoubleRow)
                    sg = pc2.tile([128, CHUNK], BF16, tag="sg")
                    nc.scalar.activation(sg[:], gps[:], AF.Sigmoid,
                                         bias=ct["cg_col"][:], scale=1.0 / F8S)
                    fps = pf.tile([128, CHUNK], F32, tag="fps")
                    nc.tensor.matmul(fps[:], lhsT=ct["W2_blk"][:], rhs=rz[:],
                                     start=True, stop=False)
                    nc.tensor.matmul(fps[:], lhsT=ct["negI"][:],
                                     rhs=base_h[:, sl], start=False, stop=True)
                    m2 = pc2.tile([128, CHUNK], BF16, tag="m2")
                    if meta["has_b2"]:
                        nc.vector.scalar_tensor_tensor(
                            m2[:], fps[:], ct["b2_blk"][:], sg[:],
                            op0=OP.add, op1=OP.mult)
                    else:
                        nc.vector.tensor_tensor(m2[:], fps[:], sg[:], op=OP.mult)
                    nc.gpsimd.tensor_tensor(out_t[:, sl], m2[:], base_h[:, sl],
                                            op=OP.add)

                nc.sync.dma_start(outT[i, :, :], out_t[:])

    if split_waits:
        _split_multi_waits(nc)
    return nc


def kernel(**inputs):
    global LAST_EXEC_NS
    qr = np.asarray(inputs["query_relations"]).astype(np.int32).reshape(B)
    et = np.asarray(inputs["edge_type"]).astype(np.int32).reshape(B, E)
    base = np.asarray(inputs["base_relation_reprs"], dtype=np.float32).reshape(B, R, H)
    noise = np.asarray(inputs["init_noise"], dtype=np.float32).reshape(B, R, H)
    w = {k: np.asarray(inputs[k], dtype=np.float32) for k in
         ("msg_W", "msg_b", "upd_W", "upd_b", "ln_g", "ln_b",
          "fus_W1", "fus_b1", "fus_W2", "fus_b2", "gate_W", "gate_b")}

    consts, meta = _consts(w)
    nc = _build_program(meta)

    in_maps = []
    for c in range(N_CORES):
        s = slice(c * SPC, (c + 1) * SPC)
        baseTf = np.ascontiguousarray(
            base[s].transpose(0, 2, 1)).reshape(PAIRS, 128, R)
        ec = _edge_consts(qr[s], et[s], meta)
        # DoubleRow interleaved rhs: k-subtile 0 = base, 1 = one-hot pad
        rhsI = np.empty((PAIRS, 128, 2, R), np.float32)
        rhsI[:, :, 0, :] = baseTf
        rhsI[:, :, 1, :] = ec.pop("ohpad")
        im = {
            "baseT": _bf(baseTf),
            "rhsI": _f8(rhsI),
            "noise": np.ascontiguousarray(noise[s]).reshape(SPC * R, H),
        }
        im.update(_pack_consts({**consts, **ec}))
        in_maps.append(im)

    res = run_bass_kernel_spmd(nc, in_maps, core_ids=list(range(N_CORES)),
                               trace=PROFILE)
    LAST_EXEC_NS = res.exec_time_ns

    out = np.empty((B, R, H), np.float32)
    for c in range(N_CORES):
        o = np.asarray(res.results[c]["outT"], dtype=np.float32).reshape(SPC, H, R)
        out[c * SPC:(c + 1) * SPC] = o.transpose(0, 2, 1)
    return out
